# revision 1
# baseline (speedup 1.0000x reference)
"""HQQ+SVD quantized linear for TRN2, tensor-parallel over out_features on 8 cores.

Math (matches reference.py):
  W_f = (w_int - zp)*scale + svd_up @ svd_down          [OUT, IN]
  scale_w[o] = max_k |W_f[o,k]| / 127
  W_q8 = round(W_f / scale_w)  (stored +1536 in fp16 so the fp16 cast rounds RNE)
  x_q  = round(x / scale_x), scale_x = rowmax|x|/127    (host, exact fp32 ops)
  acc'[o,t] = sum_k (W_q8+1536)[o,k] * x_q[t,k]         (fp16 PE matmul, exact ints)
  out[t,o] = (acc' - 1536*sum_k x_q[t,k]) * scale_x[t] * scale_w[o] + bias[o]
"""
import sys
sys.path.insert(0, "/opt/trn_rl_repo")

import numpy as np
import concourse.bass as bass
import concourse.bacc as bacc
import concourse.tile as tile
import concourse.mybir as mybir

F32 = mybir.dt.float32
F32R = mybir.dt.float32r
F16 = mybir.dt.float16
I32 = mybir.dt.int32
ALU = mybir.AluOpType
ACTF = mybir.ActivationFunctionType
AX = mybir.AxisListType

OUT, N_GROUPS, GROUP = 11008, 32, 128
IN = N_GROUPS * GROUP
RANK = 128
T = 64
NCORES = 8
SHARD = OUT // NCORES            # 1376
PAD = 1408                       # 11 * 128
NTILES = PAD // 128              # 11
C_MAGIC = 1536.0
C_BIG = 12582912.0  # 1.5*2^23: fp32 RNE-to-integer magic
INV127 = np.float32(1.0) / np.float32(127.0)

_nc_cache = {}


def _build():
    if "nc" in _nc_cache:
        return _nc_cache["nc"]
    nc = bacc.Bacc("TRN2", target_bir_lowering=False, debug=False)

    w_d = nc.dram_tensor("w", [PAD, IN], I32, kind="ExternalInput")
    zp_d = nc.dram_tensor("zp", [PAD, N_GROUPS], F32, kind="ExternalInput")
    sc_d = nc.dram_tensor("sc", [PAD, N_GROUPS], F32, kind="ExternalInput")
    upT_d = nc.dram_tensor("upT", [RANK, PAD], F32, kind="ExternalInput")
    down_d = nc.dram_tensor("down", [RANK, IN], F32, kind="ExternalInput")
    bias_d = nc.dram_tensor("bias", [PAD, 1], F32, kind="ExternalInput")
    xqt_d = nc.dram_tensor("xqt", [IN, T], F16, kind="ExternalInput")
    sxb_d = nc.dram_tensor("sxb", [128, T], F32, kind="ExternalInput")
    vb_d = nc.dram_tensor("vb", [128, T], F32, kind="ExternalInput")
    id_d = nc.dram_tensor("ident", [128, 128], F16, kind="ExternalInput")
    out_d = nc.dram_tensor("out", [PAD, T], F32, kind="ExternalOutput")

    with tile.TileContext(nc) as tc:
        with (
            tc.tile_pool(name="const", bufs=1) as cp,
            tc.tile_pool(name="work", bufs=2) as wp,
            tc.tile_pool(name="ps", bufs=2, space="PSUM") as ps,
        ):
            # ---- phase 0: constants
            id_t = cp.tile([128, 128], F16, tag="id")
            nc.sync.dma_start(out=id_t[:], in_=id_d[:])
            sxb_t = cp.tile([128, T], F32, tag="sxb")
            nc.sync.dma_start(out=sxb_t[:], in_=sxb_d[:])
            vb_t = cp.tile([128, T], F32, tag="vb")
            nc.sync.dma_start(out=vb_t[:], in_=vb_d[:])

            xqt_t = cp.tile([128, N_GROUPS * T], F16, tag="xqt")
            for c in range(N_GROUPS):
                nc.sync.dma_start(out=xqt_t[:, c * T:(c + 1) * T],
                                  in_=xqt_d[c * 128:(c + 1) * 128, :])

            tmp32 = wp.tile([128, IN], F32, tag="deq")
            nc.sync.dma_start(out=tmp32[:], in_=down_d[:])
            down_r = cp.tile([128, IN], F32R, tag="downr")
            nc.vector.tensor_copy(down_r[:], tmp32[:])

            tmp32b = wp.tile([128, IN], F32, tag="deq")
            nc.sync.dma_start(out=tmp32b[:, :PAD], in_=upT_d[:])
            upT_r = cp.tile([128, PAD], F32R, tag="upr")
            nc.vector.tensor_copy(upT_r[:], tmp32b[:, :PAD])

            # ---- per out-tile pipeline
            for i in range(NTILES):
                osl = slice(i * 128, (i + 1) * 128)
                wt = wp.tile([128, IN], I32, tag="wt")
                nc.sync.dma_start(out=wt[:], in_=w_d[osl, :])
                zp_t = wp.tile([128, N_GROUPS], F32, tag="zpt")
                nc.sync.dma_start(out=zp_t[:], in_=zp_d[osl, :])
                sc_t = wp.tile([128, N_GROUPS], F32, tag="sct")
                nc.sync.dma_start(out=sc_t[:], in_=sc_d[osl, :])
                bias_t = wp.tile([128, 1], F32, tag="bt")
                nc.sync.dma_start(out=bias_t[:], in_=bias_d[osl, :])

                # dequant (DVE): (w - zp)*scale per group
                deq = wp.tile([128, IN], F32, tag="deq")
                for g in range(N_GROUPS):
                    gs = slice(g * GROUP, (g + 1) * GROUP)
                    nc.vector.tensor_scalar(
                        deq[:, gs], wt[:, gs], zp_t[:, g:g + 1], sc_t[:, g:g + 1],
                        op0=ALU.subtract, op1=ALU.mult)

                # svd quarter + add quarter
                wf = wp.tile([128, IN], F32, tag="wf")
                for q in range(4):
                    qs = slice(q * 1024, (q + 1) * 1024)
                    cq = ps.tile([128, 1024], F32, tag="svd")
                    for h in range(2):
                        hs = slice(h * 512, (h + 1) * 512)
                        nc.tensor.matmul(
                            cq[:, hs], upT_r[:, osl],
                            down_r[:, q * 1024 + h * 512: q * 1024 + (h + 1) * 512],
                            start=True, stop=True)
                    nc.vector.tensor_tensor(wf[:, qs], deq[:, qs], cq[:],
                                            ALU.add)

                # absmax -> scales
                aabs = wp.tile([128, 1], F32, tag="aabs")
                nc.vector.tensor_reduce(aabs[:], wf[:], axis=AX.X, op=ALU.max,
                                        apply_absolute_value=True)
                rec = wp.tile([128, 1], F32, tag="rec")
                nc.vector.reciprocal(rec[:], aabs[:])
                r127 = wp.tile([128, 1], F32, tag="r127")
                nc.vector.tensor_scalar(r127[:], rec[:], 127.0, None, op0=ALU.mult)
                sw = wp.tile([128, 1], F32, tag="sw")
                nc.vector.tensor_scalar(sw[:], aabs[:], float(INV127), None,
                                        op0=ALU.mult)

                # requant on ACT: fp16 cast rounds RNE to integer via +1536
                q16 = wp.tile([128, IN], F16, tag="q16")
                nc.scalar.activation(q16[:], wf[:], ACTF.Copy,
                                     bias=C_MAGIC, scale=r127[:, 0:1])

                # transpose chunks via plain matmul: chunk.T @ I -> psum fp32
                # 4 chunks per psum bank, one ACT copy per bank
                wtT = wp.tile([128, IN], F16, tag="wtT")
                for b in range(8):
                    ptr = ps.tile([128, 512], F32, tag="ptr")
                    for j in range(4):
                        c = b * 4 + j
                        nc.tensor.matmul(ptr[:, j * 128:(j + 1) * 128],
                                         q16[:, c * 128:(c + 1) * 128],
                                         id_t[:], start=True, stop=True,
                                         skip_group_check=True)
                    nc.scalar.copy(wtT[:, b * 512:(b + 1) * 512], ptr[:])

                # main matmul: acc[o, t] += WT_c.T @ xqT_c
                acc = ps.tile([128, T], F32, tag="acc")
                for c in range(N_GROUPS):
                    nc.tensor.matmul(acc[:], wtT[:, c * 128:(c + 1) * 128],
                                     xqt_t[:, c * T:(c + 1) * T],
                                     start=(c == 0), stop=(c == 31))

                # epilogue: ((acc*sxB) - vB)*sw + bias
                e1 = wp.tile([128, T], F32, tag="e1")
                nc.vector.tensor_tensor(e1[:], acc[:], sxb_t[:], ALU.mult)
                e2 = wp.tile([128, T], F32, tag="e2")
                nc.vector.tensor_tensor(e2[:], e1[:], vb_t[:], ALU.subtract)
                e3 = wp.tile([128, T], F32, tag="e3")
                nc.vector.tensor_scalar(e3[:], e2[:], sw[:, 0:1], bias_t[:, 0:1],
                                        op0=ALU.mult, op1=ALU.add)
                nc.sync.dma_start(out=out_d[osl, :], in_=e3[:])

    nc.compile()
    _nc_cache["nc"] = nc
    return nc


def kernel(x, weight, scale, zero_point, svd_up, svd_down, bias):
    x = np.asarray(x)
    weight = np.asarray(weight)
    scale = np.asarray(scale)
    zero_point = np.asarray(zero_point)
    svd_up = np.asarray(svd_up)
    svd_down = np.asarray(svd_down)
    bias = np.asarray(bias)

    # ---- host x-quant (exact fp32 ops as in reference)
    xt = x.reshape(-1, IN).astype(np.float32)
    scale_x = (np.max(np.abs(xt), axis=1, keepdims=True)
               / np.float32(127.0)).astype(np.float32)          # [T,1]
    x_q = np.clip(np.round(xt / scale_x), -128, 127).astype(np.float32)
    xqT = np.ascontiguousarray(x_q.T).astype(np.float16)        # [IN, T]
    s_t = x_q.sum(axis=1).astype(np.float32)                    # [T]
    sxb = np.broadcast_to(scale_x[:, 0][None, :], (128, T)).astype(np.float32)
    vb = np.broadcast_to((np.float32(C_MAGIC) * s_t * scale_x[:, 0])[None, :],
                         (128, T)).astype(np.float32)
    ident = np.eye(128, dtype=np.float16)

    nc = _build()

    in_maps = []
    npad = PAD - SHARD
    for c in range(NCORES):
        sl = slice(c * SHARD, (c + 1) * SHARD)
        w_c = np.concatenate(
            [weight[sl].reshape(SHARD, IN),
             np.ones((npad, IN), np.int32)], axis=0).astype(np.int32)
        zp_c = np.concatenate(
            [zero_point[sl], np.zeros((npad, N_GROUPS), np.float32)],
            axis=0).astype(np.float32)
        sc_c = np.concatenate(
            [scale[sl], np.ones((npad, N_GROUPS), np.float32)],
            axis=0).astype(np.float32)
        upT_c = np.concatenate(
            [svd_up[sl].T, np.zeros((RANK, npad), np.float32)],
            axis=1).astype(np.float32)
        upT_c = np.ascontiguousarray(upT_c)
        bias_c = np.concatenate(
            [bias[sl], np.zeros(npad, np.float32)]).astype(np.float32)
        in_maps.append(dict(
            w=w_c, zp=zp_c, sc=sc_c, upT=upT_c,
            down=svd_down.astype(np.float32), bias=bias_c.reshape(PAD, 1),
            xqt=xqT, sxb=sxb, vb=vb, ident=ident))

    _nc_cache["last_in_maps"] = in_maps
    from concourse.bass_utils import run_bass_kernel_spmd
    res = run_bass_kernel_spmd(nc, in_maps, core_ids=list(range(NCORES)))
    outs = [r["out"][:SHARD] for r in res.results]              # [SHARD, T] each
    full = np.concatenate(outs, axis=0)                         # [OUT, T]
    return np.ascontiguousarray(full.T)[None].astype(np.float32)  # [1, T, OUT]

